# revision 2
# baseline (speedup 1.0000x reference)
"""Trainium2 Bass kernel for windowed multi-head attention with conv QKV (v5).

Three launches:
  L1: qkv conv, sharded by (batch, w-quarter) across 8 cores.
  L2: windowed attention; host pre-assembles the reference's buggy-stride
      padded flat k/v storage and per-core strip/q slices.
  L3: output 3x3 conv, sharded by (batch, block-pair).
"""

import numpy as np
import ml_dtypes
import concourse.bass as bass
import concourse.bacc as bacc
import concourse.mybir as mybir
from concourse.tile import TileContext
from concourse.bass_utils import run_bass_kernel_spmd

F32 = mybir.dt.float32
F32R = mybir.dt.float32r
BF16 = mybir.dt.bfloat16
I16 = mybir.dt.int16
AF = mybir.ActivationFunctionType
ALU = mybir.AluOpType

NCORES = 8
B, CIN, H, W = 2, 64, 32, 192
DM, NH, CH = 32, 8, 4
QS, FL, F = 24, 8, 40
M = W // QS
PB = H * W
W2 = W + 2 * FL                 # 208
S_N, S_C, S_H = CH * H * W, H * W, W   # buggy strides 24576/6144/192
KSLICE = 7 * S_N + 3 * S_C + 31 * S_H + QS + F   # 196624
HQ = H * QS                     # 768
NKC = 10
WQ = 48                         # w-quarter width
WS = WQ + 2                     # x slice width with halo
WRS = WS + 2                    # 52 staging raster row width
NSTS = (H + 2) * WRS            # 1768
SCH_A = 128.0 / np.log(2.0)
SCH_B = 127.0 * 128.0 - 7.42

_CACHE = {}


def _sap(tile, p0, npart, off, dims):
    b0 = tile[:]
    ps = int(b0.ap[0][0])
    return bass.AP(b0.tensor, b0.offset + p0 * ps + off, [[ps, npart]] + dims)


def _build_l1():
    """qkv conv over a [CIN, H, WS] x slice -> yout [96, H*WQ] (bf16)."""
    nc = bacc.Bacc(None, target_bir_lowering=False, debug=False,
                   num_devices=NCORES)
    xs = nc.dram_tensor("xs", [CIN, H, WS], BF16, kind="ExternalInput").ap()
    wblob = nc.dram_tensor("wblob", [128, 582], BF16,
                           kind="ExternalInput").ap()
    onesrow = nc.dram_tensor("onesrow", [1, NSTS], BF16,
                             kind="ExternalInput").ap()
    yout = nc.dram_tensor("yout", [96, H * WQ], BF16,
                          kind="ExternalOutput").ap()

    with TileContext(nc) as tc:
        with (
            tc.tile_pool(name="sb", bufs=1) as sb,
            tc.tile_pool(name="ps", bufs=3, space="PSUM") as ps,
        ):
            wb = sb.tile([128, 582], BF16, tag="wb", name="wb")
            nc.sync.dma_start(out=wb[:], in_=wblob[:])
            X2 = sb.tile([128, NSTS], BF16, tag="X2", name="X2")
            X1 = sb.tile([65, NSTS], BF16, tag="X1", name="X1")
            nc.sync.dma_start(out=X1[64:65, :], in_=onesrow[:])
            x_sb = sb.tile([64, H * WS], BF16, tag="x_sb", name="x_sb")
            yo = sb.tile([96, H * WQ], BF16, tag="yo", name="yo")

            wA = [wb[:, dy * 97:dy * 97 + 96] for dy in range(3)]
            wB = [wb[0:65, 291 + dy * 97:291 + dy * 97 + 96]
                  for dy in range(3)]

            engs = [nc.scalar, nc.vector, nc.gpsimd]
            planes = ((X2, 0, 0), (X2, 64, 1), (X1, 0, 2))
            for pi, (dst, p0, s) in enumerate(planes):
                j0 = max(0, 1 - s)
                j1 = min(WRS, WS + 1 - s)
                nc.vector.memset(
                    _sap(dst, p0, 64, 0, [[(H + 1) * WRS, 2], [1, WRS]]),
                    0.0)
                if j0 > 0:
                    nc.vector.memset(
                        _sap(dst, p0, 64, WRS, [[WRS, H], [1, j0]]), 0.0)
                if j1 < WRS:
                    nc.gpsimd.memset(
                        _sap(dst, p0, 64, WRS + j1,
                             [[WRS, H], [1, WRS - j1]]), 0.0)

            def _stage(hc):
                nc.sync.dma_start(
                    out=x_sb[:, hc * 8 * WS:(hc + 1) * 8 * WS],
                    in_=bass.AP(xs.tensor, xs.offset + hc * 8 * WS,
                                [[H * WS, 64], [1, 8 * WS]]))
                for pi, (dst, p0, s) in enumerate(planes):
                    j0 = max(0, 1 - s)
                    j1 = min(WRS, WS + 1 - s)
                    eng = engs[(pi + hc) % 3]
                    dsta = _sap(dst, p0, 64, (1 + hc * 8) * WRS + j0,
                                [[WRS, 8], [1, j1 - j0]])
                    srca = _sap(x_sb, 0, 64, hc * 8 * WS + (j0 - 1 + s),
                                [[WS, 8], [1, j1 - j0]])
                    if eng is nc.scalar:
                        eng.copy(dsta, srca)
                    else:
                        eng.tensor_copy(dsta, srca)

            def _conv(hc):
                yp = ps.tile([96, 384], F32, tag="yp", name="yp")
                h0 = hc * 8
                for dy in range(3):
                    off = (h0 + dy) * WRS + 1
                    nc.tensor.matmul(
                        yp[:], wA[dy],
                        _sap(X2, 0, 128, off, [[WRS, 8], [1, WQ]]),
                        start=(dy == 0), stop=False)
                    nc.tensor.matmul(
                        yp[:], wB[dy],
                        _sap(X1, 0, 65, off, [[WRS, 8], [1, WQ]]),
                        start=False, stop=(dy == 2))
                eng = [nc.scalar, nc.vector][hc % 2]
                dsty = yo[:, h0 * WQ:(h0 + 8) * WQ]
                if eng is nc.scalar:
                    eng.copy(dsty, yp[:])
                else:
                    eng.tensor_copy(dsty, yp[:])
                nc.sync.dma_start(
                    out=bass.AP(yout.tensor, yout.offset + h0 * WQ,
                                [[H * WQ, 96], [1, 8 * WQ]]),
                    in_=yo[:, h0 * WQ:(h0 + 8) * WQ])

            _stage(0)
            _stage(1)
            _conv(0)
            _stage(2)
            _conv(1)
            _stage(3)
            _conv(2)
            _conv(3)
    nc.finalize()
    return nc


def _build_l2():
    """Pure attention for one (batch, block-pair): 16 groups."""
    nc = bacc.Bacc(None, target_bir_lowering=False, debug=False,
                   num_devices=NCORES)
    kin = nc.dram_tensor("kin", [KSLICE], BF16, kind="ExternalInput").ap()
    vin = nc.dram_tensor("vin", [KSLICE], BF16, kind="ExternalInput").ap()
    qdin = nc.dram_tensor("qdin", [DM, H * 2 * QS], BF16,
                          kind="ExternalInput").ap()
    id4 = nc.dram_tensor("id4", [4, 4], BF16, kind="ExternalInput").ap()
    o_out = nc.dram_tensor("o_out", [128, 384], F32,
                           kind="ExternalOutput").ap()

    with TileContext(nc) as tc:
        with (
            tc.tile_pool(name="qdp", bufs=1) as qdp,
            tc.tile_pool(name="ga", bufs=4) as ga,
            tc.tile_pool(name="ste", bufs=24) as stp,
            tc.tile_pool(name="vep", bufs=2) as vep,
            tc.tile_pool(name="on", bufs=2) as on,
            tc.tile_pool(name="stps", bufs=3, space="PSUM") as stps,
            tc.tile_pool(name="ops", bufs=1, space="PSUM") as ops,
        ):
            id4_sb = qdp.tile([4, 4], BF16, tag="id4", name="id4")
            nc.sync.dma_start(out=id4_sb[:], in_=id4[:])
            o_all = on.tile([128, 384], F32, tag="o_all", name="o_all",
                            bufs=1)
            # 4 rotating 52-col regions: [0,30) accum, f32-cols [32,52) vt
            oacc = ops.tile([128, 1024], F32, tag="oacc", name="oacc")
            opstep = int(oacc[:].ap[0][0])

            qds = [qdp.tile([4, H * 2 * QS], BF16, tag=f"qd{n}",
                            name=f"qd{n}") for n in range(NH)]
            qd_loaded = set()

            def _load_qd(n):
                if n in qd_loaded or n >= NH:
                    return
                qd_loaded.add(n)
                nc.sync.dma_start(
                    out=qds[n][:],
                    in_=bass.AP(qdin.tensor,
                                qdin.offset + 4 * n * H * 2 * QS,
                                [[H * 2 * QS, 4], [1, H * 2 * QS]]))

            ve_sb = [vep.tile([128, 5 * NKC], BF16, tag=f"ve{i}",
                              name=f"ve{i}") for i in range(3)]
            for t in ve_sb:
                nc.vector.memset(_sap(t, 0, 128, 4, [[5, NKC], [1, 1]]),
                                 1.0)

            k2s, v2s = {}, {}

            def _densify(g):
                n, mm = g // 2, g % 2
                k2 = ga.tile([4, NKC * 128], BF16, tag="k2", name="k2")
                v2 = ga.tile([4, NKC * 128], BF16, tag="v2", name="v2")
                k2s[g], v2s[g] = k2, v2
                for (dstt, src) in ((k2, kin), (v2, vin)):
                    nc.sync.dma_start(
                        out=dstt[:],
                        in_=bass.AP(src.tensor,
                                    src.offset + n * S_N + mm * QS,
                                    [[S_C, 4], [S_H, H], [1, F]]))

            def _norm(g_):
                base = (g_ % 2) * 512
                rec = on.tile([128, 6], F32, tag="rec", name="rec")
                nc.vector.reciprocal(
                    rec[:], bass.AP(oacc[:].tensor,
                                    oacc[:].offset + base + 4,
                                    [[opstep, 128], [5, 6]]))
                rep = on.tile([128, 24], F32, tag="rep", name="rep")
                nc.vector.tensor_copy(
                    rep[:], bass.AP(rec[:].tensor, rec[:].offset,
                                    [[int(rec[:].ap[0][0]), 128], [1, 6],
                                     [0, 4]]))
                nc.vector.tensor_mul(
                    o_all[:, g_ * 24:(g_ + 1) * 24],
                    bass.AP(oacc[:].tensor, oacc[:].offset + base,
                            [[opstep, 128], [5, 6], [1, 4]]),
                    rep[:])

            ecost = {0: 825.0, 1: 925.0}
            eload = {0: 0.0, 1: 0.0}

            def pick_engine():
                e = min(ecost, key=lambda e_: eload[e_] + ecost[e_])
                eload[e] += ecost[e]
                return e

            _densify(0)
            _load_qd(0)
            _densify(1)
            _load_qd(1)
            stes = {}

            def emit_pv_group(g_):
                # 6 sequential accumulation chains (one at a time per bank)
                base = (g_ % 2) * 512
                ve = ve_sb[g_ % 3]
                tiles = stes.pop(g_)
                for s in range(6):
                    for c_ in range(NKC):
                        nc.tensor.matmul(
                            oacc[:, base + s * 5:base + s * 5 + 5],
                            tiles[c_][:, s * 128:(s + 1) * 128],
                            ve[:, c_ * 5:(c_ + 1) * 5],
                            start=(c_ == 0), stop=(c_ == NKC - 1),
                            skip_group_check=True)

            def _prep_group(g_):
                # transposes + ve copy for group g_ (v2 -> vt -> ve);
                # vt sits in oacc bank (g_%2) outside cols [0,30); written
                # only while that bank has no open accumulation group
                v2 = v2s.pop(g_)
                base = (g_ % 2) * 512
                ve = ve_sb[g_ % 3]
                vt_bf = oacc[:, base + 64:base + 84].bitcast(BF16)
                for cc in range(NKC):
                    nc.tensor.transpose(
                        vt_bf[:, cc * 4:(cc + 1) * 4],
                        v2[:, cc * 128:(cc + 1) * 128], id4_sb[:])
                nc.scalar.copy(
                    _sap(ve, 0, 128, 0, [[5, NKC], [1, 4]]),
                    bass.AP(vt_bf.tensor, vt_bf.offset,
                            [[int(vt_bf.ap[0][0]), 128], [4, NKC],
                             [1, 4]]))
                eload[0] += 220.0

            stream = [(g, c) for g in range(NH * 2) for c in range(NKC)]
            norm_due = {g_ * 10 + 14: g_ for g_ in range(NH * 2)}
            _prep_group(0)
            for i, (g, c) in enumerate(stream):
                n, mm = g // 2, g % 2
                if c == 0 and g >= 1:
                    emit_pv_group(g - 1)
                gn = norm_due.get(i)
                if gn is not None:
                    _norm(gn)
                    if gn in (5, 9, 13):
                        g0 = gn - 5
                        nc.sync.dma_start(
                            out=bass.AP(o_out.tensor, o_out.offset + g0 * 24,
                                        [[384, 128], [1, 96]]),
                            in_=o_all[:, g0 * 24:(g0 + 4) * 24])
                if c == 1 and g + 2 < NH * 2:
                    _densify(g + 2)
                    _load_qd((g + 4) // 2)
                if c == 2 and g + 1 < NH * 2:
                    _prep_group(g + 1)
                k2 = k2s[g]
                qd = qds[n]
                st = stps.tile([128, 1024], F32, tag="st", name="st")
                for qh in range(2):
                    nc.tensor.matmul(
                        st[:, qh * 512:qh * 512 + 384],
                        k2[:, c * 128:(c + 1) * 128],
                        _sap(qd, 0, 4, mm * QS + qh * 16 * 2 * QS,
                             [[2 * QS, 16], [1, QS]]),
                        start=True, stop=True, skip_group_check=True)
                ste = stp.tile([128, 768], BF16, tag="ste", name="ste")
                sin = bass.AP(st[:].tensor, st[:].offset,
                              [[int(st[:].ap[0][0]), 128], [512, 2],
                               [1, 384]])
                kind = pick_engine()
                if kind == 0:
                    nc.scalar.activation(ste[:], sin, AF.Exp)
                else:
                    nc.vector.tensor_scalar(ste[:].bitcast(I16), sin,
                                            SCH_A, SCH_B, ALU.mult, ALU.add)
                stes.setdefault(g, []).append(ste)
                if c == NKC - 1:
                    k2s.pop(g)
                    eload[1] += 660.0   # full norm chain on DVE
            emit_pv_group(NH * 2 - 1)
            for j in sorted(norm_due):
                if j >= len(stream):
                    _norm(norm_due[j])
            nc.sync.dma_start(
                out=bass.AP(o_out.tensor, o_out.offset + 12 * 24,
                            [[384, 128], [1, 96]]),
                in_=o_all[:, 12 * 24:16 * 24])
    nc.finalize()
    return nc


def _build_l3():
    nc = bacc.Bacc(None, target_bir_lowering=False, debug=False,
                   num_devices=NCORES)
    WO = 2 * QS
    oh3 = nc.dram_tensor("oh3", [96, (H + 2) * WO], F32R,
                         kind="ExternalInput").ap()
    w2 = nc.dram_tensor("w2", [96, 192], F32R, kind="ExternalInput").ap()
    out = nc.dram_tensor("out", [64, H * WO], F32, kind="ExternalOutput").ap()

    with TileContext(nc) as tc:
        with (
            tc.tile_pool(name="sb", bufs=1) as sb,
            tc.tile_pool(name="ps", bufs=2, space="PSUM") as ps,
        ):
            w2_sb = sb.tile([96, 192], F32R, tag="w2", name="w2sb")
            nc.sync.dma_start(out=w2_sb[:], in_=w2[:])
            osb = sb.tile([96, (H + 2) * WO], F32R, tag="osb", name="osb")
            for hh in range(4):
                r0 = hh * 9
                r1 = min(H + 2, r0 + 9)
                nc.sync.dma_start(
                    out=osb[:, r0 * WO:r1 * WO],
                    in_=bass.AP(oh3.tensor, oh3.offset + r0 * WO,
                                [[(H + 2) * WO, 96], [1, (r1 - r0) * WO]]))
            ot = sb.tile([64, H * WO], F32, tag="ot", name="ot")
            hsz = [10, 10, 10, 2]
            h0 = 0
            for hi, hn in enumerate(hsz):
                nt = hn * WO
                yp = ps.tile([64, 512], F32, tag="yp", name="yp")
                for dy in range(3):
                    off = (h0 + dy) * WO
                    nc.tensor.matmul(
                        yp[:, 0:nt], w2_sb[:, dy * 64:(dy + 1) * 64],
                        _sap(osb, 0, 96, off, [[1, nt]]),
                        start=(dy == 0), stop=(dy == 2))
                eng = [nc.scalar, nc.vector][hi % 2]
                if eng is nc.scalar:
                    eng.copy(ot[:, h0 * WO:h0 * WO + nt], yp[:, 0:nt])
                else:
                    eng.tensor_copy(ot[:, h0 * WO:h0 * WO + nt], yp[:, 0:nt])
                nc.sync.dma_start(
                    out=bass.AP(out.tensor, out.offset + h0 * WO,
                                [[H * WO, 64], [1, nt]]),
                    in_=ot[:, h0 * WO:h0 * WO + nt])
                h0 += hn
    nc.finalize()
    return nc


def _prep_wblob(q_w, q_b, k_w, k_b, v_w, v_b):
    sc = CH ** -0.5
    q_w = q_w * sc
    q_b = q_b * sc
    Wc = np.concatenate([q_w, k_w, v_w], axis=0)
    bc = np.concatenate([q_b, k_b, v_b], axis=0)
    blob = np.zeros((128, 582), np.float32)
    for dy in range(3):
        blob[0:64, dy * 97:dy * 97 + 96] = Wc[:, :, dy, 0].T
        blob[64:128, dy * 97:dy * 97 + 96] = Wc[:, :, dy, 1].T
        blob[0:64, 291 + dy * 97:291 + dy * 97 + 96] = Wc[:, :, dy, 2].T
        if dy == 1:
            blob[64, 291 + dy * 97:291 + dy * 97 + 96] = bc
    return blob.astype(ml_dtypes.bfloat16)


def kernel(x, q_w, q_b, k_w, k_b, v_w, v_b, out_w):
    x = np.asarray(x, np.float32)
    if "l1" not in _CACHE:
        _CACHE["l1"] = _build_l1()
        _CACHE["l2"] = _build_l2()
        _CACHE["l3"] = _build_l3()
    nc1, nc2, nc3 = _CACHE["l1"], _CACHE["l2"], _CACHE["l3"]

    wblob = _prep_wblob(
        np.asarray(q_w, np.float32), np.asarray(q_b, np.float32),
        np.asarray(k_w, np.float32), np.asarray(k_b, np.float32),
        np.asarray(v_w, np.float32), np.asarray(v_b, np.float32))
    onesrow = np.ones((1, NSTS), ml_dtypes.bfloat16)
    xbf = x.astype(ml_dtypes.bfloat16)
    # L1: core k = (b=k//4, wq=k%4): x slice [64, 32, 50] (1-col halo,
    # zero-padded at the global edges)
    xpad = np.zeros((B, CIN, H, W + 2), ml_dtypes.bfloat16)
    xpad[:, :, :, 1:W + 1] = xbf
    in1 = []
    for k in range(NCORES):
        b, wq = k // 4, k % 4
        in1.append({"xs": np.ascontiguousarray(
            xpad[b, :, :, wq * WQ:wq * WQ + WS]),
            "wblob": wblob, "onesrow": onesrow})
    res1 = run_bass_kernel_spmd(nc1, in1, list(range(NCORES)))

    yfull = np.zeros((B, 96, H, W), ml_dtypes.bfloat16)
    for k in range(NCORES):
        b, wq = k // 4, k % 4
        yfull[b, :, :, wq * WQ:(wq + 1) * WQ] = \
            res1.results[k]["yout"].reshape(96, H, WQ)

    # padded flat storages with the buggy-stride layout
    kpad = np.zeros((B, DM, H, W2), ml_dtypes.bfloat16)
    vpad = np.zeros((B, DM, H, W2), ml_dtypes.bfloat16)
    kpad[:, :, :, FL:FL + W] = yfull[:, 32:64]
    vpad[:, :, :, FL:FL + W] = yfull[:, 64:96]
    kflat = kpad.reshape(-1)
    vflat = vpad.reshape(-1)

    id4 = np.eye(4, dtype=ml_dtypes.bfloat16)
    in2 = []
    for k in range(NCORES):
        b, m0 = k // 4, 2 * (k % 4)
        cb = b * NH * S_N + m0 * QS
        qd = np.ascontiguousarray(
            yfull[b, 0:32, :, m0 * QS:(m0 + 2) * QS]).reshape(DM, -1)
        in2.append({"kin": np.ascontiguousarray(kflat[cb:cb + KSLICE]),
                    "vin": np.ascontiguousarray(vflat[cb:cb + KSLICE]),
                    "qdin": qd, "id4": id4})
    res2 = run_bass_kernel_spmd(nc2, in2, list(range(NCORES)))

    o = np.zeros((B, DM, H, W), np.float32)
    for k in range(NCORES):
        b, m0 = k // 4, 2 * (k % 4)
        oo = res2.results[k]["o_out"].reshape(128, NH, 2, 6, 4)
        oo = oo.transpose(1, 2, 3, 0, 4).reshape(NH, 2, HQ, 4)
        for mm in range(2):
            blk = oo[:, mm].reshape(NH, H, QS, 4)
            o[b, :, :, (m0 + mm) * QS:(m0 + mm + 1) * QS] = (
                blk.transpose(0, 3, 1, 2).reshape(DM, H, QS))

    w2 = np.zeros((96, 192), np.float32)
    ow = np.asarray(out_w, np.float32)
    for dy in range(3):
        for dx in range(3):
            w2[dx * 32:(dx + 1) * 32, dy * 64:(dy + 1) * 64] = \
                ow[:, :, dy, dx].T
    in3 = []
    WO = 2 * QS
    for k in range(NCORES):
        b, m0 = k // 4, 2 * (k % 4)
        c0 = m0 * QS
        oh3 = np.zeros((96, H + 2, WO), np.float32)
        for dx in range(3):
            lo, hi = c0 + dx - 1, c0 + dx - 1 + WO
            slo, shi = max(0, lo), min(W, hi)
            oh3[dx * 32:(dx + 1) * 32, 1:H + 1, slo - lo:WO - (hi - shi)] = \
                o[b, :, :, slo:shi]
        in3.append({"oh3": oh3.reshape(96, -1), "w2": w2})
    res3 = run_bass_kernel_spmd(nc3, in3, list(range(NCORES)))

    out = np.zeros((B, 64, H, W), np.float32)
    for k in range(NCORES):
        b, m0 = k // 4, 2 * (k % 4)
        out[b, :, :, m0 * QS:(m0 + 2) * QS] = \
            res3.results[k]["out"].reshape(64, H, WO)
    return out


# revision 3
# speedup vs baseline: 1.0253x; 1.0253x over previous
"""Trainium2 Bass kernel for windowed multi-head attention with conv QKV (v5).

Three launches:
  L1: qkv conv, sharded by (batch, w-quarter) across 8 cores.
  L2: windowed attention; host pre-assembles the reference's buggy-stride
      padded flat k/v storage and per-core strip/q slices.
  L3: output 3x3 conv, sharded by (batch, block-pair).
"""

import numpy as np
import ml_dtypes
import concourse.bass as bass
import concourse.bacc as bacc
import concourse.mybir as mybir
from concourse.tile import TileContext
from concourse.bass_utils import run_bass_kernel_spmd

F32 = mybir.dt.float32
F32R = mybir.dt.float32r
BF16 = mybir.dt.bfloat16
I16 = mybir.dt.int16
AF = mybir.ActivationFunctionType
ALU = mybir.AluOpType

NCORES = 8
B, CIN, H, W = 2, 64, 32, 192
DM, NH, CH = 32, 8, 4
QS, FL, F = 24, 8, 40
M = W // QS
PB = H * W
W2 = W + 2 * FL                 # 208
S_N, S_C, S_H = CH * H * W, H * W, W   # buggy strides 24576/6144/192
KSLICE = 7 * S_N + 3 * S_C + 31 * S_H + QS + F   # 196624
HQ = H * QS                     # 768
NKC = 10
WQ = 48                         # w-quarter width
WS = WQ + 2                     # x slice width with halo
WRS = WS + 2                    # 52 staging raster row width
NSTS = (H + 2) * WRS            # 1768
SCH_A = 128.0 / np.log(2.0)
SCH_B = 127.0 * 128.0 - 7.42

_CACHE = {}


def _sap(tile, p0, npart, off, dims):
    b0 = tile[:]
    ps = int(b0.ap[0][0])
    return bass.AP(b0.tensor, b0.offset + p0 * ps + off, [[ps, npart]] + dims)


def _build_l1():
    """qkv conv over a [CIN, H, WS] x slice -> yout [96, H*WQ] (bf16)."""
    nc = bacc.Bacc(None, target_bir_lowering=False, debug=False,
                   num_devices=NCORES)
    xs = nc.dram_tensor("xs", [CIN, H, WS], BF16, kind="ExternalInput").ap()
    wblob = nc.dram_tensor("wblob", [128, 582], BF16,
                           kind="ExternalInput").ap()
    onesrow = nc.dram_tensor("onesrow", [1, NSTS], BF16,
                             kind="ExternalInput").ap()
    yout = nc.dram_tensor("yout", [96, H * WQ], BF16,
                          kind="ExternalOutput").ap()

    with TileContext(nc) as tc:
        with (
            tc.tile_pool(name="sb", bufs=1) as sb,
            tc.tile_pool(name="ps", bufs=3, space="PSUM") as ps,
        ):
            wb = sb.tile([128, 582], BF16, tag="wb", name="wb")
            nc.scalar.dma_start(out=wb[:], in_=wblob[:])
            X2 = sb.tile([128, NSTS], BF16, tag="X2", name="X2")
            X1 = sb.tile([65, NSTS], BF16, tag="X1", name="X1")
            nc.scalar.dma_start(out=X1[64:65, :], in_=onesrow[:])
            x_sb = sb.tile([64, H * WS], BF16, tag="x_sb", name="x_sb")
            yo = sb.tile([96, H * WQ], BF16, tag="yo", name="yo")

            wA = [wb[:, dy * 97:dy * 97 + 96] for dy in range(3)]
            wB = [wb[0:65, 291 + dy * 97:291 + dy * 97 + 96]
                  for dy in range(3)]

            engs = [nc.scalar, nc.vector, nc.gpsimd]
            planes = ((X2, 0, 0), (X2, 64, 1), (X1, 0, 2))
            for pi, (dst, p0, s) in enumerate(planes):
                j0 = max(0, 1 - s)
                j1 = min(WRS, WS + 1 - s)
                nc.vector.memset(
                    _sap(dst, p0, 64, 0, [[(H + 1) * WRS, 2], [1, WRS]]),
                    0.0)
                if j0 > 0:
                    nc.vector.memset(
                        _sap(dst, p0, 64, WRS, [[WRS, H], [1, j0]]), 0.0)
                if j1 < WRS:
                    nc.gpsimd.memset(
                        _sap(dst, p0, 64, WRS + j1,
                             [[WRS, H], [1, WRS - j1]]), 0.0)

            def _stage(hc):
                nc.sync.dma_start(
                    out=x_sb[:, hc * 8 * WS:(hc + 1) * 8 * WS],
                    in_=bass.AP(xs.tensor, xs.offset + hc * 8 * WS,
                                [[H * WS, 64], [1, 8 * WS]]))
                for pi, (dst, p0, s) in enumerate(planes):
                    j0 = max(0, 1 - s)
                    j1 = min(WRS, WS + 1 - s)
                    eng = engs[(pi + hc) % 3]
                    dsta = _sap(dst, p0, 64, (1 + hc * 8) * WRS + j0,
                                [[WRS, 8], [1, j1 - j0]])
                    srca = _sap(x_sb, 0, 64, hc * 8 * WS + (j0 - 1 + s),
                                [[WS, 8], [1, j1 - j0]])
                    if eng is nc.scalar:
                        eng.copy(dsta, srca)
                    else:
                        eng.tensor_copy(dsta, srca)

            def _conv(hc):
                yp = ps.tile([96, 384], F32, tag="yp", name="yp")
                h0 = hc * 8
                for dy in range(3):
                    off = (h0 + dy) * WRS + 1
                    nc.tensor.matmul(
                        yp[:], wA[dy],
                        _sap(X2, 0, 128, off, [[WRS, 8], [1, WQ]]),
                        start=(dy == 0), stop=False)
                    nc.tensor.matmul(
                        yp[:], wB[dy],
                        _sap(X1, 0, 65, off, [[WRS, 8], [1, WQ]]),
                        start=False, stop=(dy == 2))
                eng = [nc.scalar, nc.vector][hc % 2]
                dsty = yo[:, h0 * WQ:(h0 + 8) * WQ]
                if eng is nc.scalar:
                    eng.copy(dsty, yp[:])
                else:
                    eng.tensor_copy(dsty, yp[:])
                nc.sync.dma_start(
                    out=bass.AP(yout.tensor, yout.offset + h0 * WQ,
                                [[H * WQ, 96], [1, 8 * WQ]]),
                    in_=yo[:, h0 * WQ:(h0 + 8) * WQ])

            _stage(0)
            _stage(1)
            _conv(0)
            _stage(2)
            _conv(1)
            _stage(3)
            _conv(2)
            _conv(3)
    nc.finalize()
    return nc


def _build_l2():
    """Pure attention for one (batch, block-pair): 16 groups."""
    nc = bacc.Bacc(None, target_bir_lowering=False, debug=False,
                   num_devices=NCORES)
    kin = nc.dram_tensor("kin", [KSLICE], BF16, kind="ExternalInput").ap()
    vin = nc.dram_tensor("vin", [KSLICE], BF16, kind="ExternalInput").ap()
    qdin = nc.dram_tensor("qdin", [DM, H * 2 * QS], BF16,
                          kind="ExternalInput").ap()
    id4 = nc.dram_tensor("id4", [4, 4], BF16, kind="ExternalInput").ap()
    o_out = nc.dram_tensor("o_out", [128, 384], F32,
                           kind="ExternalOutput").ap()

    with TileContext(nc) as tc:
        with (
            tc.tile_pool(name="qdp", bufs=1) as qdp,
            tc.tile_pool(name="ga", bufs=4) as ga,
            tc.tile_pool(name="ste", bufs=24) as stp,
            tc.tile_pool(name="vep", bufs=2) as vep,
            tc.tile_pool(name="on", bufs=2) as on,
            tc.tile_pool(name="stps", bufs=3, space="PSUM") as stps,
            tc.tile_pool(name="ops", bufs=1, space="PSUM") as ops,
        ):
            id4_sb = qdp.tile([4, 4], BF16, tag="id4", name="id4")
            nc.sync.dma_start(out=id4_sb[:], in_=id4[:])
            o_all = on.tile([128, 384], F32, tag="o_all", name="o_all",
                            bufs=1)
            # 4 rotating 52-col regions: [0,30) accum, f32-cols [32,52) vt
            oacc = ops.tile([128, 1024], F32, tag="oacc", name="oacc")
            opstep = int(oacc[:].ap[0][0])

            qds = [qdp.tile([4, H * 2 * QS], BF16, tag=f"qd{n}",
                            name=f"qd{n}") for n in range(NH)]
            qd_loaded = set()

            def _load_qd(n):
                if n in qd_loaded or n >= NH:
                    return
                qd_loaded.add(n)
                nc.sync.dma_start(
                    out=qds[n][:],
                    in_=bass.AP(qdin.tensor,
                                qdin.offset + 4 * n * H * 2 * QS,
                                [[H * 2 * QS, 4], [1, H * 2 * QS]]))

            ve_sb = [vep.tile([128, 5 * NKC], BF16, tag=f"ve{i}",
                              name=f"ve{i}") for i in range(3)]
            for t in ve_sb:
                nc.vector.memset(_sap(t, 0, 128, 4, [[5, NKC], [1, 1]]),
                                 1.0)

            k2s, v2s = {}, {}

            def _densify(g):
                n, mm = g // 2, g % 2
                k2 = ga.tile([4, NKC * 128], BF16, tag="k2", name="k2")
                v2 = ga.tile([4, NKC * 128], BF16, tag="v2", name="v2")
                k2s[g], v2s[g] = k2, v2
                for (dstt, src) in ((k2, kin), (v2, vin)):
                    nc.sync.dma_start(
                        out=dstt[:],
                        in_=bass.AP(src.tensor,
                                    src.offset + n * S_N + mm * QS,
                                    [[S_C, 4], [S_H, H], [1, F]]))

            def _norm(g_):
                base = (g_ % 2) * 512
                rec = on.tile([128, 6], F32, tag="rec", name="rec")
                nc.vector.reciprocal(
                    rec[:], bass.AP(oacc[:].tensor,
                                    oacc[:].offset + base + 4,
                                    [[opstep, 128], [5, 6]]))
                rep = on.tile([128, 24], F32, tag="rep", name="rep")
                nc.vector.tensor_copy(
                    rep[:], bass.AP(rec[:].tensor, rec[:].offset,
                                    [[int(rec[:].ap[0][0]), 128], [1, 6],
                                     [0, 4]]))
                nc.vector.tensor_mul(
                    o_all[:, g_ * 24:(g_ + 1) * 24],
                    bass.AP(oacc[:].tensor, oacc[:].offset + base,
                            [[opstep, 128], [5, 6], [1, 4]]),
                    rep[:])

            ecost = {0: 825.0, 1: 925.0}
            eload = {0: 0.0, 1: 0.0}

            def pick_engine():
                e = min(ecost, key=lambda e_: eload[e_] + ecost[e_])
                eload[e] += ecost[e]
                return e

            _densify(0)
            _load_qd(0)
            _densify(1)
            _load_qd(1)
            stes = {}

            def emit_pv_group(g_):
                # 6 sequential accumulation chains (one at a time per bank)
                base = (g_ % 2) * 512
                ve = ve_sb[g_ % 3]
                tiles = stes.pop(g_)
                for s in range(6):
                    for c_ in range(NKC):
                        nc.tensor.matmul(
                            oacc[:, base + s * 5:base + s * 5 + 5],
                            tiles[c_][:, s * 128:(s + 1) * 128],
                            ve[:, c_ * 5:(c_ + 1) * 5],
                            start=(c_ == 0), stop=(c_ == NKC - 1),
                            skip_group_check=True)

            def _prep_group(g_):
                # transposes + ve copy for group g_ (v2 -> vt -> ve);
                # vt sits in oacc bank (g_%2) outside cols [0,30); written
                # only while that bank has no open accumulation group
                v2 = v2s.pop(g_)
                base = (g_ % 2) * 512
                ve = ve_sb[g_ % 3]
                vt_bf = oacc[:, base + 64:base + 84].bitcast(BF16)
                for cc in range(NKC):
                    nc.tensor.transpose(
                        vt_bf[:, cc * 4:(cc + 1) * 4],
                        v2[:, cc * 128:(cc + 1) * 128], id4_sb[:])
                nc.scalar.copy(
                    _sap(ve, 0, 128, 0, [[5, NKC], [1, 4]]),
                    bass.AP(vt_bf.tensor, vt_bf.offset,
                            [[int(vt_bf.ap[0][0]), 128], [4, NKC],
                             [1, 4]]))
                eload[0] += 220.0

            stream = [(g, c) for g in range(NH * 2) for c in range(NKC)]
            norm_due = {g_ * 10 + 14: g_ for g_ in range(NH * 2)}
            _prep_group(0)
            for i, (g, c) in enumerate(stream):
                n, mm = g // 2, g % 2
                if c == 0 and g >= 1:
                    emit_pv_group(g - 1)
                gn = norm_due.get(i)
                if gn is not None:
                    _norm(gn)
                    if gn in (5, 9, 13):
                        g0 = gn - 5
                        nc.sync.dma_start(
                            out=bass.AP(o_out.tensor, o_out.offset + g0 * 24,
                                        [[384, 128], [1, 96]]),
                            in_=o_all[:, g0 * 24:(g0 + 4) * 24])
                if c == 1 and g + 2 < NH * 2:
                    _densify(g + 2)
                    _load_qd((g + 4) // 2)
                if c == 2 and g + 1 < NH * 2:
                    _prep_group(g + 1)
                k2 = k2s[g]
                qd = qds[n]
                st = stps.tile([128, 1024], F32, tag="st", name="st")
                for qh in range(2):
                    nc.tensor.matmul(
                        st[:, qh * 512:qh * 512 + 384],
                        k2[:, c * 128:(c + 1) * 128],
                        _sap(qd, 0, 4, mm * QS + qh * 16 * 2 * QS,
                             [[2 * QS, 16], [1, QS]]),
                        start=True, stop=True, skip_group_check=True)
                ste = stp.tile([128, 768], BF16, tag="ste", name="ste")
                sin = bass.AP(st[:].tensor, st[:].offset,
                              [[int(st[:].ap[0][0]), 128], [512, 2],
                               [1, 384]])
                kind = pick_engine()
                if kind == 0:
                    nc.scalar.activation(ste[:], sin, AF.Exp)
                else:
                    nc.vector.tensor_scalar(ste[:].bitcast(I16), sin,
                                            SCH_A, SCH_B, ALU.mult, ALU.add)
                stes.setdefault(g, []).append(ste)
                if c == NKC - 1:
                    k2s.pop(g)
                    eload[1] += 660.0   # full norm chain on DVE
            emit_pv_group(NH * 2 - 1)
            for j in sorted(norm_due):
                if j >= len(stream):
                    _norm(norm_due[j])
            nc.sync.dma_start(
                out=bass.AP(o_out.tensor, o_out.offset + 12 * 24,
                            [[384, 128], [1, 96]]),
                in_=o_all[:, 12 * 24:16 * 24])
    nc.finalize()
    return nc


def _build_l3():
    nc = bacc.Bacc(None, target_bir_lowering=False, debug=False,
                   num_devices=NCORES)
    WO = 2 * QS
    oh3 = nc.dram_tensor("oh3", [96, (H + 2) * WO], F32R,
                         kind="ExternalInput").ap()
    w2 = nc.dram_tensor("w2", [96, 192], F32R, kind="ExternalInput").ap()
    out = nc.dram_tensor("out", [64, H * WO], F32, kind="ExternalOutput").ap()

    with TileContext(nc) as tc:
        with (
            tc.tile_pool(name="sb", bufs=1) as sb,
            tc.tile_pool(name="ps", bufs=4, space="PSUM") as ps,
        ):
            w2_sb = sb.tile([96, 192], F32R, tag="w2", name="w2sb")
            nc.scalar.dma_start(out=w2_sb[:], in_=w2[:])
            osb = sb.tile([96, (H + 2) * WO], F32R, tag="osb", name="osb")
            bnds = [0, 13, 23, 34]
            for hh in range(3):
                r0 = bnds[hh]
                r1 = bnds[hh + 1]
                nc.sync.dma_start(
                    out=osb[:, r0 * WO:r1 * WO],
                    in_=bass.AP(oh3.tensor, oh3.offset + r0 * WO,
                                [[(H + 2) * WO, 96], [1, (r1 - r0) * WO]]))
            ot = sb.tile([64, H * WO], F32, tag="ot", name="ot")
            hsz = [10, 10, 10, 2]
            h0 = 0
            for hi, hn in enumerate(hsz):
                nt = hn * WO
                yp = ps.tile([64, 512], F32, tag="yp", name="yp")
                for dy in range(3):
                    off = (h0 + dy) * WO
                    nc.tensor.matmul(
                        yp[:, 0:nt], w2_sb[:, dy * 64:(dy + 1) * 64],
                        _sap(osb, 0, 96, off, [[1, nt]]),
                        start=(dy == 0), stop=(dy == 2))
                eng = [nc.scalar, nc.vector][hi % 2]
                if eng is nc.scalar:
                    eng.copy(ot[:, h0 * WO:h0 * WO + nt], yp[:, 0:nt])
                else:
                    eng.tensor_copy(ot[:, h0 * WO:h0 * WO + nt], yp[:, 0:nt])
                nc.sync.dma_start(
                    out=bass.AP(out.tensor, out.offset + h0 * WO,
                                [[H * WO, 64], [1, nt]]),
                    in_=ot[:, h0 * WO:h0 * WO + nt])
                h0 += hn
    nc.finalize()
    return nc


def _prep_wblob(q_w, q_b, k_w, k_b, v_w, v_b):
    sc = CH ** -0.5
    q_w = q_w * sc
    q_b = q_b * sc
    Wc = np.concatenate([q_w, k_w, v_w], axis=0)
    bc = np.concatenate([q_b, k_b, v_b], axis=0)
    blob = np.zeros((128, 582), np.float32)
    for dy in range(3):
        blob[0:64, dy * 97:dy * 97 + 96] = Wc[:, :, dy, 0].T
        blob[64:128, dy * 97:dy * 97 + 96] = Wc[:, :, dy, 1].T
        blob[0:64, 291 + dy * 97:291 + dy * 97 + 96] = Wc[:, :, dy, 2].T
        if dy == 1:
            blob[64, 291 + dy * 97:291 + dy * 97 + 96] = bc
    return blob.astype(ml_dtypes.bfloat16)


def kernel(x, q_w, q_b, k_w, k_b, v_w, v_b, out_w):
    x = np.asarray(x, np.float32)
    if "l1" not in _CACHE:
        _CACHE["l1"] = _build_l1()
        _CACHE["l2"] = _build_l2()
        _CACHE["l3"] = _build_l3()
    nc1, nc2, nc3 = _CACHE["l1"], _CACHE["l2"], _CACHE["l3"]

    wblob = _prep_wblob(
        np.asarray(q_w, np.float32), np.asarray(q_b, np.float32),
        np.asarray(k_w, np.float32), np.asarray(k_b, np.float32),
        np.asarray(v_w, np.float32), np.asarray(v_b, np.float32))
    onesrow = np.ones((1, NSTS), ml_dtypes.bfloat16)
    xbf = x.astype(ml_dtypes.bfloat16)
    # L1: core k = (b=k//4, wq=k%4): x slice [64, 32, 50] (1-col halo,
    # zero-padded at the global edges)
    xpad = np.zeros((B, CIN, H, W + 2), ml_dtypes.bfloat16)
    xpad[:, :, :, 1:W + 1] = xbf
    in1 = []
    for k in range(NCORES):
        b, wq = k // 4, k % 4
        in1.append({"xs": np.ascontiguousarray(
            xpad[b, :, :, wq * WQ:wq * WQ + WS]),
            "wblob": wblob, "onesrow": onesrow})
    res1 = run_bass_kernel_spmd(nc1, in1, list(range(NCORES)))

    yfull = np.zeros((B, 96, H, W), ml_dtypes.bfloat16)
    for k in range(NCORES):
        b, wq = k // 4, k % 4
        yfull[b, :, :, wq * WQ:(wq + 1) * WQ] = \
            res1.results[k]["yout"].reshape(96, H, WQ)

    # padded flat storages with the buggy-stride layout
    kpad = np.zeros((B, DM, H, W2), ml_dtypes.bfloat16)
    vpad = np.zeros((B, DM, H, W2), ml_dtypes.bfloat16)
    kpad[:, :, :, FL:FL + W] = yfull[:, 32:64]
    vpad[:, :, :, FL:FL + W] = yfull[:, 64:96]
    kflat = kpad.reshape(-1)
    vflat = vpad.reshape(-1)

    id4 = np.eye(4, dtype=ml_dtypes.bfloat16)
    in2 = []
    for k in range(NCORES):
        b, m0 = k // 4, 2 * (k % 4)
        cb = b * NH * S_N + m0 * QS
        qd = np.ascontiguousarray(
            yfull[b, 0:32, :, m0 * QS:(m0 + 2) * QS]).reshape(DM, -1)
        in2.append({"kin": np.ascontiguousarray(kflat[cb:cb + KSLICE]),
                    "vin": np.ascontiguousarray(vflat[cb:cb + KSLICE]),
                    "qdin": qd, "id4": id4})
    res2 = run_bass_kernel_spmd(nc2, in2, list(range(NCORES)))

    o = np.zeros((B, DM, H, W), np.float32)
    for k in range(NCORES):
        b, m0 = k // 4, 2 * (k % 4)
        oo = res2.results[k]["o_out"].reshape(128, NH, 2, 6, 4)
        oo = oo.transpose(1, 2, 3, 0, 4).reshape(NH, 2, HQ, 4)
        for mm in range(2):
            blk = oo[:, mm].reshape(NH, H, QS, 4)
            o[b, :, :, (m0 + mm) * QS:(m0 + mm + 1) * QS] = (
                blk.transpose(0, 3, 1, 2).reshape(DM, H, QS))

    w2 = np.zeros((96, 192), np.float32)
    ow = np.asarray(out_w, np.float32)
    for dy in range(3):
        for dx in range(3):
            w2[dx * 32:(dx + 1) * 32, dy * 64:(dy + 1) * 64] = \
                ow[:, :, dy, dx].T
    in3 = []
    WO = 2 * QS
    for k in range(NCORES):
        b, m0 = k // 4, 2 * (k % 4)
        c0 = m0 * QS
        oh3 = np.zeros((96, H + 2, WO), np.float32)
        for dx in range(3):
            lo, hi = c0 + dx - 1, c0 + dx - 1 + WO
            slo, shi = max(0, lo), min(W, hi)
            oh3[dx * 32:(dx + 1) * 32, 1:H + 1, slo - lo:WO - (hi - shi)] = \
                o[b, :, :, slo:shi]
        in3.append({"oh3": oh3.reshape(96, -1), "w2": w2})
    res3 = run_bass_kernel_spmd(nc3, in3, list(range(NCORES)))

    out = np.zeros((B, 64, H, W), np.float32)
    for k in range(NCORES):
        b, m0 = k // 4, 2 * (k % 4)
        out[b, :, :, m0 * QS:(m0 + 2) * QS] = \
            res3.results[k]["out"].reshape(64, H, WO)
    return out
